# revision 14
# baseline (speedup 1.0000x reference)
"""Trainium2 Bass kernel for nn_Correlation: -mean(einsum('itj,itl->ijl', x, y)).

Math: mean over [B, C, C] of corr[b,j,l] = sum_t x[b,t,j] y[b,t,l] equals
  (1/(B*C^2)) * sum_{b,t} (sum_j x[b,t,j]) * (sum_l y[b,t,l])
so the kernel only needs per-row sums of x and y plus a dot product —
a pure memory-bound streaming reduction (no matmul).

Sharding: data-parallel over batch. 8 cores, 1 batch element each.

Schedule (per core): stream x[b] and y[b] ([2048, 1024] f32, 8 MB each)
through SBUF in chunks. x loads ride the SP HWDGE ring, y loads the ACT
HWDGE ring — separate queues so the 16 SDMA engines round-robin between
them and neither tensor's chunk completions gate the other's. x rows map
to SBUF partitions 0..123 only: SDMA engine 15 (serving partitions
92-95/124-127) is reliably ~2x slower under profiling, so it carries
only y bytes (half share) and never straggles a chunk completion. The
vector engine row-sums x chunks (free-dim tensor_reduce); the scalar
engine row-sums y per column (activation Copy with accum_out, in
place). First and last chunks are small so reduces start early and the
post-stream tail stays short. Row sums are stored via two single-wait
DMAs (y via SWDGE as soon as activations finish, x on the SP ring);
the host unscrambles, multiplies x/y row sums, sums, and scales.

Constraints honored (this walrus build allows ONE sync wait per
instruction — verified empirically, even for Drain):
- every chunk gets a dedicated SBUF slot (no WAR/WAW waits on loads);
- activation writes in place (a scratch tile's WAW reuse would add a
  second wait);
- reduces wait only their own chunk's DMA completion lane; store_x
  waits the DVE sem, store_y the ACT sem — one wait each;
- TileContext's tail drain is split into one drain per proc lane
  (_patch_tail_drain).
"""

import numpy as np

B, T, C = 8, 2048, 1024
P = 128             # SBUF partitions
N_CORES = 8

# x grid: 124 partitions x 16 cols (1984 rows) + tail col of 64 rows on
# partitions 0..63. Partition counts avoid 124..127 so SDMA engine 15
# (partitions 92-95/124-127) carries no x bytes.
XP = 124
XCHUNKS = [3, 4, 4, 4, 1]           # cols per chunk, sums to 16
XTAIL_ROWS = T - XP * 16            # 64
# y grid: full 128 partitions x 16 cols.
YCHUNKS = [1, 2, 3, 3, 3, 2, 1, 1]  # sums to 16

_CACHE = {}


def _patch_tail_drain(tile):
    """Split TileContext's kernel-tail drain into one drain per proc lane.

    The stock tail emits a single SP Drain waiting on every outstanding
    sem (DVE + ACT + each DMA completion lane); this walrus build caps
    sync waits per instruction below that, so codegen fails with "Too
    many sync wait commands". Waiting on the sems one drain at a time is
    equivalent (SP program order) and keeps every instruction at 1 wait.
    """
    import re
    import bass_rust
    from concourse.vector_clock import ScopedClock

    if getattr(tile.TileContext, "_tail_drain_split", False):
        return

    def _drain_and_barrier(self, tick_clock, wait_clock):
        ticks = [int(s) for s in re.findall(r"-?\d+",
                                            repr(tick_clock.global_clock))]
        lanes = [i for i, t in reversed(list(enumerate(ticks))) if t > 0]
        for i in lanes:
            part = bass_rust.VectorClock(
                [ticks[i] if j == i else 0 for j in range(len(ticks))])
            d = self.nc.sync.drain()
            wait_clock.add_sem_waits(d.ins, ScopedClock({None: part}))
        self.nc.all_engine_barrier()
        assert self.sems is not None
        popped = self.nc._tile_sem_poison_stack.pop()
        assert popped is self._sem_poison
        # no second barrier: the NRT postamble's full sem sweep makes any
        # clear-vs-postamble write race benign (both write zero)
        self.nc.clear_and_free_semaphores(list(self.sems.allocated().values()))

    tile.TileContext._drain_and_barrier = _drain_and_barrier
    tile.TileContext._tail_drain_split = True


def _x_chunks():
    """(row_offset, cols, parts, col_offset) per x chunk; the 64-row tail
    goes last — it is the smallest transfer, so the final chunk's
    load-completion -> reduce -> store chain is as short as possible."""
    out = []
    off = 0
    for a in XCHUNKS:
        out.append((XP * off, a, XP, off))
        off += a
    out.append((XP * 16, 1, XTAIL_ROWS, 16))
    return out


def _y_chunks():
    out = []
    off = 0
    for a in YCHUNKS:
        out.append((P * off, a, P, off))
        off += a
    return out


def _build_bass():
    import concourse.bass as bass
    import concourse.tile as tile
    from concourse import mybir

    _patch_tail_drain(tile)

    f32 = mybir.dt.float32
    # Bass.__init__ unconditionally memsets a const pool and emits an
    # all-engine barrier (~0.7 us on the measured critical path). This
    # kernel never reads the const APs, so suppress both during init.
    _ob, _om = bass.Bass.all_engine_barrier, bass.BassSharedVectorInterface.memset
    bass.Bass.all_engine_barrier = lambda self, *a, **k: None
    bass.BassSharedVectorInterface.memset = lambda self, *a, **k: None
    try:
        nc = bass.Bass()
    finally:
        bass.Bass.all_engine_barrier = _ob
        bass.BassSharedVectorInterface.memset = _om
    x = nc.dram_tensor("x", [T, C], f32, kind="ExternalInput")
    y = nc.dram_tensor("y", [T, C], f32, kind="ExternalInput")
    out_x = nc.dram_tensor("out_x", [XP, 17], f32, kind="ExternalOutput")
    out_y = nc.dram_tensor("out_y", [P, 16], f32, kind="ExternalOutput")

    with tile.TileContext(nc) as tc:
        with (
            # dedicated slot per chunk (unique tags, 1 buf each): load DMAs
            # never carry WAR/WAW waits
            tc.tile_pool(name="iox", bufs=1) as iox,
            tc.tile_pool(name="ioy", bufs=1) as ioy,
            tc.tile_pool(name="acc", bufs=1) as acc,
        ):
            sx = acc.tile([XP, 17], f32)  # [:, 0:16] + [0:64, 16] valid
            sy = acc.tile([P, 16], f32)

            # zero sx's one never-reduced hole on DVE (no overlap with any
            # reduce output, so no waits) so the single x store reads only
            # DVE-written bytes — reading unwritten SBUF makes Tile join
            # every engine's clock into the store's wait, overflowing the
            # 1-wait budget. (Engine APs must start on a 32-partition
            # boundary; 64 is.)
            nc.vector.memset(sx[XTAIL_ROWS:XP, 16:17], 0.0)

            # all load triggers first. x rides the SP HWDGE ring (6 loads,
            # lanes DMAHW0-5); y rides SWDGE from the otherwise-idle Pool
            # engine (8 loads, lanes DMASW0-7). Only 8 HWDGE completion
            # lanes exist and a 9th HWDGE DMA inherits a lane-reuse wait,
            # so keeping x loads + x stores at 8 leaves every store on a
            # fresh lane (its one wait budget goes to the data dep).
            xts, yts = [], []
            xcs, ycs = _x_chunks(), _y_chunks()
            for i in range(max(len(xcs), len(ycs))):
                if i < len(ycs):
                    roff, a, parts, coff = ycs[i]
                    yt = ioy.tile([P, a, C], f32, tag=f"yt{coff}")
                    nc.gpsimd.dma_start(
                        out=yt[:],
                        in_=y[roff:roff + parts * a, :]
                            .rearrange("(p a) c -> p a c", p=parts))
                    yts.append((coff, a, yt))
                if i < len(xcs):
                    roff, a, parts, coff = xcs[i]
                    xt = iox.tile([P, a, C], f32, tag=f"xt{coff}")
                    nc.sync.dma_start(
                        out=xt[0:parts],
                        in_=x[roff:roff + parts * a, :]
                            .rearrange("(p a) c -> p a c", p=parts))
                    xts.append((coff, a, parts, xt))

            for coff, a, parts, xt in xts:
                nc.vector.tensor_reduce(
                    out=sx[0:parts, coff:coff + a], in_=xt[0:parts],
                    axis=mybir.AxisListType.X, op=mybir.AluOpType.add,
                )  # parts is 124 (main) or 64 (tail); both start at 0
            for coff, a, yt in yts:
                for j in range(a):
                    nc.scalar.activation(
                        out=yt[:, j], in_=yt[:, j],
                        func=mybir.ActivationFunctionType.Copy,
                        accum_out=sy[:, coff + j:coff + j + 1],
                    )

            # stores take HWDGE lanes 6-7 (fresh, so the single allowed
            # wait is the data dep). store_y is issued from the ACT engine
            # right after its own last accumulator read.
            nc.sync.dma_start(out=out_x[:], in_=sx[:])
            nc.scalar.dma_start(out=out_y[:], in_=sy[:])
    return nc


def _run(x, y, trace=False):
    from concourse.bass_utils import run_bass_kernel_spmd

    if "nc" not in _CACHE:
        _CACHE["nc"] = _build_bass()
    nc = _CACHE["nc"]
    in_maps = [
        {"x": np.ascontiguousarray(x[i]), "y": np.ascontiguousarray(y[i])}
        for i in range(N_CORES)
    ]
    return run_bass_kernel_spmd(nc, in_maps, core_ids=list(range(N_CORES)),
                                trace=trace)


def _row_maps():
    """row index for each valid (partition, col) of the x/y sum tiles.
    Chunk at (row_offset, cols a, parts, col_offset) holds row
    roff + p*a + j at (p, coff + j)."""
    xm = np.full((XP, 17), -1, np.int64)
    for roff, a, parts, coff in _x_chunks():
        for j in range(a):
            xm[:parts, coff + j] = roff + np.arange(parts) * a + j
    ym = np.full((P, 16), -1, np.int64)
    for roff, a, parts, coff in _y_chunks():
        for j in range(a):
            ym[:parts, coff + j] = roff + np.arange(parts) * a + j
    return xm, ym


_XMAP, _YMAP = _row_maps()


def kernel(**inputs) -> np.ndarray:
    x = np.asarray(inputs["x"], dtype=np.float32)
    y = np.asarray(inputs["y"], dtype=np.float32)
    res = _run(x, y, trace=False)
    s = 0.0
    for r in res.results:
        sx = np.empty(T)
        sx[_XMAP[_XMAP >= 0]] = r["out_x"].astype(np.float64)[_XMAP >= 0]
        sy = np.empty(T)
        sy[_YMAP[_YMAP >= 0]] = r["out_y"].astype(np.float64)[_YMAP >= 0]
        s += (sx * sy).sum()
    return np.array(-s / (B * C * C), dtype=np.float32)


# revision 17
# speedup vs baseline: 1.1524x; 1.1524x over previous
"""Trainium2 Bass kernel for nn_Correlation: -mean(einsum('itj,itl->ijl', x, y)).

Math: mean over [B, C, C] of corr[b,j,l] = sum_t x[b,t,j] y[b,t,l] equals
  (1/(B*C^2)) * sum_{b,t} (sum_j x[b,t,j]) * (sum_l y[b,t,l])
so the kernel only needs per-row sums of x and y plus a dot product —
a pure memory-bound streaming reduction (no matmul).

Sharding: data-parallel over batch. 8 cores, 1 batch element each.

Schedule (per core): stream x[b] and y[b] ([2048, 1024] f32, 8 MB each)
through SBUF in chunks. x loads ride the SP HWDGE ring, y loads the ACT
HWDGE ring — separate queues so the 16 SDMA engines round-robin between
them and neither tensor's chunk completions gate the other's. x rows map
to SBUF partitions 0..123 only: SDMA engine 15 (serving partitions
92-95/124-127) is reliably ~2x slower under profiling, so it carries
only y bytes (half share) and never straggles a chunk completion. The
vector engine row-sums x chunks (free-dim tensor_reduce); the scalar
engine row-sums y per column (activation Copy with accum_out, in
place). First and last chunks are small so reduces start early and the
post-stream tail stays short. Row sums are stored via two single-wait
DMAs (y via SWDGE as soon as activations finish, x on the SP ring);
the host unscrambles, multiplies x/y row sums, sums, and scales.

Constraints honored (this walrus build allows ONE sync wait per
instruction — verified empirically, even for Drain):
- every chunk gets a dedicated SBUF slot (no WAR/WAW waits on loads);
- activation writes in place (a scratch tile's WAW reuse would add a
  second wait);
- reduces wait only their own chunk's DMA completion lane; store_x
  waits the DVE sem, store_y the ACT sem — one wait each;
- TileContext's tail drain is split into one drain per proc lane
  (_patch_tail_drain).
"""

import numpy as np

B, T, C = 8, 2048, 1024
P = 128             # SBUF partitions
N_CORES = 8

# x grid: 124 partitions x 16 cols (1984 rows) + tail col of 64 rows on
# partitions 0..63. Partition counts avoid 124..127 so SDMA engine 15
# (partitions 92-95/124-127) carries no x bytes.
XP = 124
XCHUNKS = [2, 3, 4, 4, 2, 1]        # cols per chunk, sums to 16
XTAIL_ROWS = T - XP * 16            # 64
# y grid: full 128 partitions x 16 cols.
YCHUNKS = [1, 2, 3, 3, 3, 2, 1, 1]  # sums to 16

_CACHE = {}


def _patch_tail_drain(tile):
    """Split TileContext's kernel-tail drain into one drain per proc lane.

    The stock tail emits a single SP Drain waiting on every outstanding
    sem (DVE + ACT + each DMA completion lane); this walrus build caps
    sync waits per instruction below that, so codegen fails with "Too
    many sync wait commands". Waiting on the sems one drain at a time is
    equivalent (SP program order) and keeps every instruction at 1 wait.
    """
    import re
    import bass_rust
    from concourse.vector_clock import ScopedClock

    if getattr(tile.TileContext, "_tail_drain_split", False):
        return

    def _drain_and_barrier(self, tick_clock, wait_clock):
        ticks = [int(s) for s in re.findall(r"-?\d+",
                                            repr(tick_clock.global_clock))]
        lanes = [i for i, t in reversed(list(enumerate(ticks))) if t > 0]
        for i in lanes:
            part = bass_rust.VectorClock(
                [ticks[i] if j == i else 0 for j in range(len(ticks))])
            d = self.nc.sync.drain()
            wait_clock.add_sem_waits(d.ins, ScopedClock({None: part}))
        self.nc.all_engine_barrier()
        assert self.sems is not None
        popped = self.nc._tile_sem_poison_stack.pop()
        assert popped is self._sem_poison
        # no second barrier: the NRT postamble's full sem sweep makes any
        # clear-vs-postamble write race benign (both write zero)
        self.nc.clear_and_free_semaphores(list(self.sems.allocated().values()))

    tile.TileContext._drain_and_barrier = _drain_and_barrier
    tile.TileContext._tail_drain_split = True


def _x_chunks():
    """(row_offset, cols, parts, col_offset) per x chunk; the 64-row tail
    goes last — it is the smallest transfer, so the final chunk's
    load-completion -> reduce -> store chain is as short as possible."""
    out = []
    off = 0
    for a in XCHUNKS:
        out.append((XP * off, a, XP, off))
        off += a
    out.append((XP * 16, 1, XTAIL_ROWS, 16))
    return out


def _y_chunks():
    out = []
    off = 0
    for a in YCHUNKS:
        out.append((P * off, a, P, off))
        off += a
    return out


def _build_bass():
    import concourse.bass as bass
    import concourse.tile as tile
    from concourse import mybir

    _patch_tail_drain(tile)

    f32 = mybir.dt.float32
    # Bass.__init__ unconditionally memsets a const pool and emits an
    # all-engine barrier (~0.7 us on the measured critical path). This
    # kernel never reads the const APs, so suppress both during init.
    _ob, _om = bass.Bass.all_engine_barrier, bass.BassSharedVectorInterface.memset
    bass.Bass.all_engine_barrier = lambda self, *a, **k: None
    bass.BassSharedVectorInterface.memset = lambda self, *a, **k: None
    try:
        nc = bass.Bass()
    finally:
        bass.Bass.all_engine_barrier = _ob
        bass.BassSharedVectorInterface.memset = _om
    x = nc.dram_tensor("x", [T, C], f32, kind="ExternalInput")
    y = nc.dram_tensor("y", [T, C], f32, kind="ExternalInput")
    out_x = nc.dram_tensor("out_x", [XP, 17], f32, kind="ExternalOutput")
    out_y = nc.dram_tensor("out_y", [P, 16], f32, kind="ExternalOutput")

    with tile.TileContext(nc) as tc:
        with (
            # dedicated slot per chunk (unique tags, 1 buf each): load DMAs
            # never carry WAR/WAW waits
            tc.tile_pool(name="iox", bufs=1) as iox,
            tc.tile_pool(name="ioy", bufs=1) as ioy,
            tc.tile_pool(name="acc", bufs=1) as acc,
        ):
            sx = acc.tile([XP, 17], f32)  # [:, 0:16] + [0:64, 16] valid
            sy = acc.tile([P, 16], f32)

            # zero sx's one never-reduced hole on DVE (no overlap with any
            # reduce output, so no waits) so the single x store reads only
            # DVE-written bytes — reading unwritten SBUF makes Tile join
            # every engine's clock into the store's wait, overflowing the
            # 1-wait budget. (Engine APs must start on a 32-partition
            # boundary; 64 is.)
            nc.vector.memset(sx[XTAIL_ROWS:XP, 16:17], 0.0)

            # all load triggers first, ALL on the SP HWDGE ring (one ring
            # saturates the 16 SDMA engines; SWDGE bulk loads are Q7
            # descriptor-gen bound at ~86 GB/s — measured). Loads beyond
            # the 8 HWDGE completion lanes inherit a lane-reuse wait,
            # which is legal (a load has no other wait) and harmless: the
            # ring is FIFO, so the reused lane's prior DMA is long done.
            # Interleave y/x so both reduce engines are fed from the start.
            xts, yts = [], []
            xcs, ycs = _x_chunks(), _y_chunks()
            for i in range(max(len(xcs), len(ycs))):
                if i < len(ycs):
                    roff, a, parts, coff = ycs[i]
                    yt = ioy.tile([P, a, C], f32, tag=f"yt{coff}")
                    nc.sync.dma_start(
                        out=yt[:],
                        in_=y[roff:roff + parts * a, :]
                            .rearrange("(p a) c -> p a c", p=parts))
                    yts.append((coff, a, yt))
                if i < len(xcs):
                    roff, a, parts, coff = xcs[i]
                    xt = iox.tile([P, a, C], f32, tag=f"xt{coff}")
                    nc.sync.dma_start(
                        out=xt[0:parts],
                        in_=x[roff:roff + parts * a, :]
                            .rearrange("(p a) c -> p a c", p=parts))
                    xts.append((coff, a, parts, xt))

            for coff, a, parts, xt in xts:
                nc.vector.tensor_reduce(
                    out=sx[0:parts, coff:coff + a], in_=xt[0:parts],
                    axis=mybir.AxisListType.X, op=mybir.AluOpType.add,
                )  # parts is 124 (main) or 64 (tail); both start at 0
            for coff, a, yt in yts:
                for j in range(a):
                    nc.scalar.activation(
                        out=yt[:, j], in_=yt[:, j],
                        func=mybir.ActivationFunctionType.Copy,
                        accum_out=sy[:, coff + j:coff + j + 1],
                    )

            # both stores go SWDGE from the idle Pool engine: every HWDGE
            # lane is reused by then (a lane-reuse wait plus the data wait
            # would exceed the 1-wait budget), while the SWDGE lanes are
            # fresh — each store's single wait is its data dep. SWDGE
            # descriptor-gen cost is fine for these 8 KB transfers.
            nc.gpsimd.dma_start(out=out_y[:], in_=sy[:])
            nc.gpsimd.dma_start(out=out_x[:], in_=sx[:])
    return nc


def _run(x, y, trace=False):
    from concourse.bass_utils import run_bass_kernel_spmd

    if "nc" not in _CACHE:
        _CACHE["nc"] = _build_bass()
    nc = _CACHE["nc"]
    in_maps = [
        {"x": np.ascontiguousarray(x[i]), "y": np.ascontiguousarray(y[i])}
        for i in range(N_CORES)
    ]
    return run_bass_kernel_spmd(nc, in_maps, core_ids=list(range(N_CORES)),
                                trace=trace)


def _row_maps():
    """row index for each valid (partition, col) of the x/y sum tiles.
    Chunk at (row_offset, cols a, parts, col_offset) holds row
    roff + p*a + j at (p, coff + j)."""
    xm = np.full((XP, 17), -1, np.int64)
    for roff, a, parts, coff in _x_chunks():
        for j in range(a):
            xm[:parts, coff + j] = roff + np.arange(parts) * a + j
    ym = np.full((P, 16), -1, np.int64)
    for roff, a, parts, coff in _y_chunks():
        for j in range(a):
            ym[:parts, coff + j] = roff + np.arange(parts) * a + j
    return xm, ym


_XMAP, _YMAP = _row_maps()


def kernel(**inputs) -> np.ndarray:
    x = np.asarray(inputs["x"], dtype=np.float32)
    y = np.asarray(inputs["y"], dtype=np.float32)
    res = _run(x, y, trace=False)
    s = 0.0
    for r in res.results:
        sx = np.empty(T)
        sx[_XMAP[_XMAP >= 0]] = r["out_x"].astype(np.float64)[_XMAP >= 0]
        sy = np.empty(T)
        sy[_YMAP[_YMAP >= 0]] = r["out_y"].astype(np.float64)[_YMAP >= 0]
        s += (sx * sy).sum()
    return np.array(-s / (B * C * C), dtype=np.float32)


# revision 18
# speedup vs baseline: 1.6805x; 1.4583x over previous
"""Trainium2 Bass kernel for nn_Correlation: -mean(einsum('itj,itl->ijl', x, y)).

Math: mean over [B, C, C] of corr[b,j,l] = sum_t x[b,t,j] y[b,t,l] equals
  (1/(B*C^2)) * sum_{b,t} (sum_j x[b,t,j]) * (sum_l y[b,t,l])
so the kernel only needs per-row sums of x and y plus a dot product —
a pure memory-bound streaming reduction (no matmul).

Sharding: data-parallel over batch. 8 cores, 1 batch element each.

Schedule (per core): stream x[b] and y[b] ([2048, 1024] f32, 8 MB each)
through SBUF in descending-size chunks on the SP HWDGE ring. HWDGE deals
a DMA's partitions to SDMA engines in equal contiguous groups using the
largest divisor of the partition count <= 16 (measured: 128 -> 16
engines x 8, 124 -> 4 x 31, 120 -> 15 x 8). SDMA engine 15 is reliably
~25% slower under profiling and lags every full-width chunk completion
by ~10 us, so ALL bulk chunks use 120 partitions: 15 healthy engines x
~27 GB/s = ~406 GB/s with engine 15 idle. Rows = 120x17 + an 8-row
tail chunk (engines 0-7). The vector engine row-sums x chunks (free-dim
tensor_reduce); the scalar engine row-sums y per column (activation
Copy with accum_out, in place). Chunks descend in size so reduces start
early and the last load->reduce->store chain is short. Row sums are
stored via four SWDGE stores (fresh DMASW lanes, one data-dep wait
each) issued from the otherwise-idle Pool engine; the host unscrambles,
multiplies x/y row sums, sums, and scales.

Constraints honored (this walrus build allows ONE sync wait per
instruction — verified empirically, even for Drain):
- every chunk gets a dedicated SBUF slot (no WAR/WAW waits on loads);
- loads carry at most a completion-lane-reuse wait (there are only 8
  HWDGE lanes; a reused lane's prior DMA is long finished by FIFO);
- activation writes in place (a scratch tile's WAW reuse would add a
  second wait);
- stores go SWDGE so each sits on a fresh DMASW lane and spends its
  single wait on the DVE/ACT data dep;
- stores read only engine-written bytes (reading never-written SBUF
  makes Tile join every engine's clock into the wait);
- TileContext's tail drain is split into one drain per proc lane
  (_patch_tail_drain).
"""

import numpy as np

B, T, C = 8, 2048, 1024
P = 128             # SBUF partitions
N_CORES = 8

# both grids: 120 partitions x 17 cols (2040 rows) + 8-row tail col on
# partitions 0..7. 120 deals to 15 SDMA engines, skipping slow engine 15.
GP = 120
NCOLS = 17
TAIL_ROWS = T - GP * NCOLS          # 8
CHUNKS = [5, 4, 3, 2, 1, 1, 1]      # cols per chunk, sums to 17

_CACHE = {}


def _patch_tail_drain(tile):
    """Split TileContext's kernel-tail drain into one drain per proc lane.

    The stock tail emits a single SP Drain waiting on every outstanding
    sem (DVE + ACT + each DMA completion lane); this walrus build caps
    sync waits per instruction below that, so codegen fails with "Too
    many sync wait commands". Waiting on the sems one drain at a time is
    equivalent (SP program order) and keeps every instruction at 1 wait.
    """
    import re
    import bass_rust
    from concourse.vector_clock import ScopedClock

    if getattr(tile.TileContext, "_tail_drain_split", False):
        return

    def _drain_and_barrier(self, tick_clock, wait_clock):
        ticks = [int(s) for s in re.findall(r"-?\d+",
                                            repr(tick_clock.global_clock))]
        lanes = [i for i, t in reversed(list(enumerate(ticks))) if t > 0]
        for i in lanes:
            part = bass_rust.VectorClock(
                [ticks[i] if j == i else 0 for j in range(len(ticks))])
            d = self.nc.sync.drain()
            wait_clock.add_sem_waits(d.ins, ScopedClock({None: part}))
        self.nc.all_engine_barrier()
        assert self.sems is not None
        popped = self.nc._tile_sem_poison_stack.pop()
        assert popped is self._sem_poison
        # no second barrier: the NRT postamble's full sem sweep makes any
        # clear-vs-postamble write race benign (both write zero)
        self.nc.clear_and_free_semaphores(list(self.sems.allocated().values()))

    tile.TileContext._drain_and_barrier = _drain_and_barrier
    tile.TileContext._tail_drain_split = True


def _chunks():
    """(row_offset, cols, parts, col_offset) per chunk; descending sizes,
    8-row tail last so the final load->reduce->store chain is short."""
    out = []
    off = 0
    for a in CHUNKS:
        out.append((GP * off, a, GP, off))
        off += a
    out.append((GP * NCOLS, 1, TAIL_ROWS, NCOLS))
    return out


def _build_bass():
    import concourse.bass as bass
    import concourse.tile as tile
    from concourse import mybir

    _patch_tail_drain(tile)

    f32 = mybir.dt.float32
    # Bass.__init__ unconditionally memsets a const pool and emits an
    # all-engine barrier (~0.7 us on the measured critical path). This
    # kernel never reads the const APs, so suppress both during init.
    _ob, _om = bass.Bass.all_engine_barrier, bass.BassSharedVectorInterface.memset
    bass.Bass.all_engine_barrier = lambda self, *a, **k: None
    bass.BassSharedVectorInterface.memset = lambda self, *a, **k: None
    try:
        nc = bass.Bass()
    finally:
        bass.Bass.all_engine_barrier = _ob
        bass.BassSharedVectorInterface.memset = _om
    x = nc.dram_tensor("x", [T, C], f32, kind="ExternalInput")
    y = nc.dram_tensor("y", [T, C], f32, kind="ExternalInput")
    out_x = nc.dram_tensor("out_x", [GP, NCOLS + 1], f32, kind="ExternalOutput")
    out_y = nc.dram_tensor("out_y", [GP, NCOLS + 1], f32, kind="ExternalOutput")

    with tile.TileContext(nc) as tc:
        with (
            # dedicated slot per chunk (unique tags, 1 buf each): load DMAs
            # never carry WAR/WAW waits
            tc.tile_pool(name="iox", bufs=1) as iox,
            tc.tile_pool(name="ioy", bufs=1) as ioy,
            tc.tile_pool(name="acc", bufs=1) as acc,
        ):
            sx = acc.tile([GP, NCOLS + 1], f32)  # [:, :17] + [0:8, 17] valid
            sy = acc.tile([GP, NCOLS + 1], f32)

            # all load triggers first, all on the SP HWDGE ring (one ring
            # keeps per-engine FIFO order and the 15 dealt engines
            # saturated; the issuing engine has nothing else to do).
            # y chunk before x chunk at each size so the scalar engine's
            # slower per-column reduce starts as early as possible.
            xts, yts = [], []
            for roff, a, parts, coff in _chunks():
                yt = ioy.tile([P, a, C], f32, tag=f"yt{coff}")
                nc.sync.dma_start(
                    out=yt[0:parts],
                    in_=y[roff:roff + parts * a, :]
                        .rearrange("(p a) c -> p a c", p=parts))
                yts.append((coff, a, parts, yt))
                xt = iox.tile([P, a, C], f32, tag=f"xt{coff}")
                nc.sync.dma_start(
                    out=xt[0:parts],
                    in_=x[roff:roff + parts * a, :]
                        .rearrange("(p a) c -> p a c", p=parts))
                xts.append((coff, a, parts, xt))

            for coff, a, parts, xt in xts:
                nc.vector.tensor_reduce(
                    out=sx[0:parts, coff:coff + a], in_=xt[0:parts],
                    axis=mybir.AxisListType.X, op=mybir.AluOpType.add,
                )
            for coff, a, parts, yt in yts:
                for j in range(a):
                    nc.scalar.activation(
                        out=yt[0:parts, j], in_=yt[0:parts, j],
                        func=mybir.ActivationFunctionType.Copy,
                        accum_out=sy[0:parts, coff + j:coff + j + 1],
                    )

            # four SWDGE stores from the idle Pool engine, each on a fresh
            # DMASW lane with its single wait spent on the data dep; main
            # stores fire before the tails' reduces even finish. Split so
            # no store reads the never-written [8:120, 17] corner.
            nc.gpsimd.dma_start(out=out_y[:, 0:NCOLS], in_=sy[:, 0:NCOLS])
            nc.gpsimd.dma_start(out=out_x[:, 0:NCOLS], in_=sx[:, 0:NCOLS])
            nc.gpsimd.dma_start(out=out_y[0:TAIL_ROWS, NCOLS:],
                                in_=sy[0:TAIL_ROWS, NCOLS:])
            nc.gpsimd.dma_start(out=out_x[0:TAIL_ROWS, NCOLS:],
                                in_=sx[0:TAIL_ROWS, NCOLS:])
    return nc


def _run(x, y, trace=False):
    from concourse.bass_utils import run_bass_kernel_spmd

    if "nc" not in _CACHE:
        _CACHE["nc"] = _build_bass()
    nc = _CACHE["nc"]
    in_maps = [
        {"x": np.ascontiguousarray(x[i]), "y": np.ascontiguousarray(y[i])}
        for i in range(N_CORES)
    ]
    return run_bass_kernel_spmd(nc, in_maps, core_ids=list(range(N_CORES)),
                                trace=trace)


def _row_map():
    """row index for each valid (partition, col) of the sum tiles.
    Chunk at (row_offset, cols a, parts, col_offset) holds row
    roff + p*a + j at (p, coff + j)."""
    m = np.full((GP, NCOLS + 1), -1, np.int64)
    for roff, a, parts, coff in _chunks():
        for j in range(a):
            m[:parts, coff + j] = roff + np.arange(parts) * a + j
    return m


_MAP = _row_map()


def kernel(**inputs) -> np.ndarray:
    x = np.asarray(inputs["x"], dtype=np.float32)
    y = np.asarray(inputs["y"], dtype=np.float32)
    res = _run(x, y, trace=False)
    s = 0.0
    valid = _MAP >= 0
    for r in res.results:
        sx = np.empty(T)
        sx[_MAP[valid]] = r["out_x"].astype(np.float64)[valid]
        sy = np.empty(T)
        sy[_MAP[valid]] = r["out_y"].astype(np.float64)[valid]
        s += (sx * sy).sum()
    return np.array(-s / (B * C * C), dtype=np.float32)
